# revision 49
# baseline (speedup 1.0000x reference)
"""
Multi-head masked (causal) attention on 8 Trainium2 NeuronCores.

Sharding: core = 2*b + g  (b = batch 0..3, g = head-group 0..1, 6 heads each).
Each core computes, for its batch b and heads [6g, 6g+6):
    q,k,v projections -> causal attention -> partial out-projection
    (rows [384g, 384g+384) of Wo), output written TRANSPOSED [768, S] bf16.
Host gathers: out[b] = (part[2b] + part[2b+1]).T + bo.

All matmuls in bf16 (PE 1 col/cycle @2.4GHz), fp32 PSUM accumulation.
Scores are computed transposed (S^T[sk, sq] = K^T x Q^T) so:
  - exp runs on ACT straight out of PSUM (scale=1/8 fused),
  - AV uses V as the stationary operand with an appended ones-column,
    yielding ctx^T[j, sq] AND the softmax denominator in one accumulation,
  - ctx^T is exactly the lhsT layout the out-projection needs.

Attention runs in BT=512 query tiles (4 per pair): every score block then
fits one gathered [128, 2, 512] PSUM tile, so the two heads' K=64 score
matmuls are adjacent single-tile writes and DUAL-ISSUE on PE row-groups
0/64 (emitting them into separate tiles lets the Tile scheduler split the
pair and serialize them — measured 215ns apart instead of 4ns), and each
ik needs exactly one <=512-col AV matmul per head.  One exp per ik covers
both heads.  Causal structure is block-exact: for key-block ik only
sq >= 128*ik is computed; diagonal 128x128 blocks are masked by one DVE
multiply with a lower-triangular 0/1 tile after the exp.

Call order is t2-major ((p,t2) = (0,0),(1,0),(2,0),(0,1)...), so the first
call only needs q/k for pair 0 cols [0,512) -> the exp stream starts at
~13us instead of ~24us.  ACT (exp) is the binding engine in late groups
(2*SumW*0.81ns + 280ns/instr ~= 118us total), so the projection/out-proj
"filler" units are weighted toward the late ACT-bound calls to keep PE
busy there; DMA arrival order (sync: wq0,wk0,bq,bk,tri,wv,bv,wq1,wk1,
wq2,wk2,e6,e7; scalar: e0a,e0b,e1..e5,wo) matches first-use order.

Out-projection accumulates all 3 head-pairs into one PSUM tile per
128-row block and stores bf16 [768, S]; cols [1536,2048) overlap the last
call via pair-0/1 staging (even n) and plain 3-pair tiles (odd n) so the
tail after the last AV is ~5us.
"""

import numpy as np
import ml_dtypes

import concourse.bass as bass
import concourse.mybir as mybir
import concourse.tile as tile
from concourse import bacc

BF16 = mybir.dt.bfloat16
F32 = mybir.dt.float32
F8 = mybir.dt.float8e4          # e4m3

# Problem constants (hardcoded per contract)
B, S, D = 4, 2048, 768
N_HEADS_TOTAL = 12
HD = 64                      # head dim
H = 6                        # local heads per core
NPAIR = H // 2               # head pairs (Q/K computed 2 heads at a time)
NC_D = D // 128              # contraction chunks over D (6)
NSK = S // 128               # key blocks (16)
BT = 512                     # query-tile width for the attention phase
NT2 = S // BT                # query tiles (4)
VW = H * (HD + 64)           # v storage: per head [v(64) | ones(64)] (768)
SCALE = 1.0 / np.sqrt(HD)


def build_nc():
    nc = bacc.Bacc(None, target_bir_lowering=False)

    # x^T packed by 256-col chunk: row e*128+p holds [c, col] contiguous
    # (3KB per partition line -> ~18x fewer DMA descriptors than 512B lines)
    xT_d = nc.declare_dram_parameter("xT", [8 * 128, NC_D * 256], BF16,
                                     isOutput=False)
    # fp8 copy of x^T for the Q/K projections (DoubleRow): row eb*128+p
    # holds [C(3), j(2), col(512)] where d = C*256 + j*128 + p
    xq8_d = nc.declare_dram_parameter("xq8", [4 * 128, 3 * 2 * 512], F8,
                                      isOutput=False)
    # per-pair packed fp8 q/k weights: rows [p*128,(p+1)*128) are pair p as
    # [r, (C j m)] with d = C*256 + j*128 + r
    wq8_d = nc.declare_dram_parameter("wq8", [NPAIR * 128, 3 * 2 * 128], F8,
                                      isOutput=False)
    wk8_d = nc.declare_dram_parameter("wk8", [NPAIR * 128, 3 * 2 * 128], F8,
                                      isOutput=False)
    wv_d = nc.declare_dram_parameter("wv", [128, NC_D * H * HD], BF16,
                                     isOutput=False)
    bq_d = nc.declare_dram_parameter("bq", [128, NPAIR], F32, isOutput=False)
    bk_d = nc.declare_dram_parameter("bk", [128, NPAIR], F32, isOutput=False)
    bv_d = nc.declare_dram_parameter("bv", [1, H * HD], F32, isOutput=False)
    wo_d = nc.declare_dram_parameter("wo", [128, NPAIR * D], BF16,
                                     isOutput=False)
    # tri[sk, sq] = 1 where sq >= sk: multiplied into the exp'd diagonal
    # S^T block on DVE (cheaper than a PE mask matmul)
    tri_d = nc.declare_dram_parameter("tri", [128, 128], BF16,
                                      isOutput=False)
    outT_d = nc.declare_dram_parameter("outT", [D, S], BF16, isOutput=True)

    with tile.TileContext(nc) as tc:
        with (
            tc.tile_pool(name="const", bufs=1) as constp,
            tc.tile_pool(name="big", bufs=1) as bigp,
            tc.tile_pool(name="epool", bufs=6) as epool,
            tc.tile_pool(name="rpool", bufs=2) as rpool,
            tc.tile_pool(name="opool", bufs=3) as opool,
            tc.tile_pool(name="work", bufs=2, space="PSUM") as work,
            tc.tile_pool(name="ctxp", bufs=4, space="PSUM") as ctxp,
        ):
            # ---- fp8 x^T (q/k projections) on the scalar HWDGE queue first:
            # eb0 lands in two pieces so the first matmul gates on ~260KB
            xq8_sb = bigp.tile([128, 4, 3, 2, 512], F8)
            xT_sb = bigp.tile([128, 8, NC_D, 256], BF16)
            for (C0, Cn) in ((0, 2), (2, 1)):
                nc.scalar.dma_start(
                    xq8_sb[:, 0, C0:C0 + Cn].rearrange("p c j n -> p (c j n)"),
                    xq8_d[0:128, C0 * 1024:(C0 + Cn) * 1024],
                )
            for eb in range(1, 4):
                nc.scalar.dma_start(
                    xq8_sb[:, eb].rearrange("p c j n -> p (c j n)"),
                    xq8_d[eb * 128:(eb + 1) * 128, :],
                )
            # preload the Exp activation table while DMAs stream (issued
            # after the xT DMAs so it doesn't delay them on this queue)
            scr0 = constp.tile([128, 1], F32)
            scr1 = constp.tile([128, 1], F32)
            nc.vector.memset(scr0[:], 0.0)
            nc.scalar.activation(scr1[:], scr0[:],
                                 mybir.ActivationFunctionType.Exp, scale=1.0)
            # ---- weights on the sync HWDGE queue, in first-use order
            wq8_sb = constp.tile([128, NPAIR, 3, 2, 128], F8)
            wk8_sb = constp.tile([128, NPAIR, 3, 2, 128], F8)
            nc.sync.dma_start(wq8_sb[:, 0].rearrange("p c j n -> p (c j n)"),
                              wq8_d[0:128, :])
            nc.sync.dma_start(wk8_sb[:, 0].rearrange("p c j n -> p (c j n)"),
                              wk8_d[0:128, :])
            bq_sb = constp.tile([128, NPAIR], F32)
            nc.sync.dma_start(bq_sb[:], bq_d[:])
            bk_sb = constp.tile([128, NPAIR], F32)
            nc.sync.dma_start(bk_sb[:], bk_d[:])
            tri_sb = constp.tile([128, 128], BF16)
            nc.sync.dma_start(tri_sb[:], tri_d[:])
            wv_sb = constp.tile([128, NC_D, H * HD], BF16)
            nc.sync.dma_start(wv_sb[:].rearrange("p c n -> p (c n)"), wv_d[:])
            nc.sync.dma_start(
                xT_sb[:, 0].rearrange("p c n -> p (c n)"), xT_d[0:128, :])
            bvb_sb = constp.tile([128, H * HD], F32)
            nc.sync.dma_start(
                bvb_sb[:, None, :],
                bv_d[:].partition_broadcast(128),
            )
            nc.sync.dma_start(
                xT_sb[:, 1].rearrange("p c n -> p (c n)"),
                xT_d[128:256, :])
            # remaining loads interleaved by first-use time on the sync
            # queue (the scalar queue carries NO trigger after the exp
            # stream starts — a blocked trigger stalls every exp behind it
            # in ACT program order)
            nc.sync.dma_start(
                xT_sb[:, 2].rearrange("p c n -> p (c n)"), xT_d[256:384, :])
            for p in (1, 2):
                nc.sync.dma_start(
                    wq8_sb[:, p].rearrange("p_ c j n -> p_ (c j n)"),
                    wq8_d[p * 128:(p + 1) * 128, :])
                nc.sync.dma_start(
                    wk8_sb[:, p].rearrange("p_ c j n -> p_ (c j n)"),
                    wk8_d[p * 128:(p + 1) * 128, :])
            nc.sync.dma_start(
                xT_sb[:, 3].rearrange("p c n -> p (c n)"), xT_d[384:512, :])
            wo_sb = constp.tile([128, NPAIR, D], BF16)
            nc.sync.dma_start(wo_sb[:].rearrange("p c n -> p (c n)"), wo_d[:])
            for e in range(4, 8):
                nc.sync.dma_start(
                    xT_sb[:, e].rearrange("p c n -> p (c n)"),
                    xT_d[e * 128:(e + 1) * 128, :],
                )

            qT_sb = bigp.tile([128, NPAIR, S], BF16)
            kT_sb = bigp.tile([128, NPAIR, S], BF16)
            v_sb = bigp.tile([128, NSK, VW], BF16)
            ctxT_sb = bigp.tile([128, NPAIR, S], BF16)
            stage_sb = bigp.tile([128, 6, 512], F32)


            def qk_tile(p, which, t, n0=0, nw=512):
                # fp8 DoubleRow: 3 matmuls of K=256 (two 128-subtiles in the
                # free dim), each streaming nw cols at 0.5 cycles/col
                dst_sb, w_sb, b_sb = ((qT_sb, wq8_sb, bq_sb),
                                      (kT_sb, wk8_sb, bk_sb))[which]
                ps = work.tile([128, 1024], F32, tag="work")
                for C in range(3):
                    nc.tensor.matmul(
                        ps[:, 0:nw],
                        w_sb[:, p, C],
                        xq8_sb[:, t, C, :, n0:n0 + nw],
                        start=(C == 0), stop=(C == 2),
                        perf_mode=mybir.MatmulPerfMode.DoubleRow,
                    )
                nc.vector.tensor_add(
                    out=dst_sb[:, p, t * 512 + n0:t * 512 + n0 + nw],
                    in0=ps[:, 0:nw],
                    in1=b_sb[:, p:p + 1].broadcast_to((128, nw)),
                )

            def v_proj(s):
                ps = work.tile([128, 1024], F32, tag="work")
                for c in range(NC_D):
                    nc.tensor.matmul(
                        ps[:, 0:H * HD],
                        xT_sb[:, s // 2, c,
                              (s % 2) * 128:(s % 2) * 128 + 128],
                        wv_sb[:, c, :],
                        start=(c == 0), stop=(c == NC_D - 1),
                    )
                # this block's ones-columns (the denominator trick) ride
                # along here so the one big startup memset doesn't stall
                # the DVE bias-add chain
                nc.vector.memset(
                    v_sb[:, s, :].rearrange("p (h c) -> p h c", h=H)[:, :, HD:128],
                    1.0,
                )
                nc.vector.tensor_add(
                    out=v_sb[:, s, :].rearrange("p (h c) -> p h c", h=H)[:, :, 0:HD],
                    in0=ps[:, 0:H * HD].rearrange("p (h c) -> p h c", h=H),
                    in1=bvb_sb.rearrange("p (h c) -> p h c", h=H)[:, :, 0:HD],
                )

            def attention(p, t2, fillers=None, late_fillers=None):
                # Both heads of pair p for query cols [512*t2, 512*(t2+1)).
                # Per ik: one gathered [128, 2, W] score tile (the two K=64
                # matmuls dual-issue on row-groups 0/64), one exp, one
                # <=512-col AV matmul per head.  Software-pipelined: scores
                # and exp of ik+1 are emitted BEFORE the AV matmuls of ik.
                q0 = t2 * BT
                nik = (q0 + BT) // 128
                ctxs = [ctxp.tile([128, BT], F32, tag="ctx",
                                  name=f"ctx{p}{t2}{hf}") for hf in range(2)]

                def emit_scores(ik):
                    sq0 = max(q0, 128 * ik)
                    W = q0 + BT - sq0
                    s_ps = work.tile([128, 1024], F32, tag="work",
                                     name="spsm")
                    for half in range(2):
                        hp = slice(half * 64, half * 64 + 64)
                        nc.tensor.matmul(
                            s_ps[:, half * 512:half * 512 + W],
                            kT_sb[hp, p, ik * 128:(ik + 1) * 128],
                            qT_sb[hp, p, sq0:sq0 + W],
                            start=True, stop=True,
                            skip_group_check=True,
                        )
                    e_sb = epool.tile([128, 1024], BF16, tag="e", name="em")
                    nc.scalar.activation(
                        e_sb[:].rearrange("p (g n) -> p g n", g=2)[:, :, 0:W],
                        s_ps[:].rearrange("p (g n) -> p g n", g=2)[:, :, 0:W],
                        mybir.ActivationFunctionType.Exp, scale=float(SCALE),
                    )
                    if 128 * ik >= q0:
                        # diagonal block at cols [0,128): zero the causally-
                        # masked entries (exp'd real scores).  On GPSIMD:
                        # it is idle, while DVE bursts (norm/bias) would
                        # delay this exp->mask->AV critical path
                        for half in range(2):
                            nc.gpsimd.tensor_mul(
                                e_sb[:, half * 512:half * 512 + 128],
                                e_sb[:, half * 512:half * 512 + 128],
                                tri_sb[:])
                    return [e_sb[:, 0:W], e_sb[:, 512:512 + W]]

                # lookahead 2: ACT always has a spare score tile queued, so
                # it can run back-to-back exps instead of pacing 1/PE-step
                es_q = [emit_scores(ik) for ik in range(min(2, nik))]
                for ik in range(nik):
                    es = es_q.pop(0)
                    if ik + 2 < nik:
                        es_q.append(emit_scores(ik + 2))
                    if fillers:
                        # spread remaining fillers evenly over remaining iks
                        npop = -(-len(fillers) // (nik - ik))
                        for _ in range(npop):
                            fillers.pop(0)()
                    if late_fillers and ik >= nik - 4:
                        # units reserved for the last iks (keeps PE warm
                        # through the final normalization window)
                        npop = -(-len(late_fillers) // (nik - ik))
                        for _ in range(npop):
                            late_fillers.pop(0)()
                    off = max(q0, 128 * ik) - q0
                    for half in range(2):
                        h = 2 * p + half
                        nc.tensor.matmul(
                            ctxs[half][:, off:BT],
                            v_sb[:, ik, h * 128:(h + 1) * 128],
                            es[half],
                            start=(ik == 0), stop=(ik == nik - 1),
                            skip_group_check=True,
                        )
                # normalization is deferred: the caller emits norm(ctxs) as a
                # filler of the NEXT call, so the boundary DVE burst doesn't
                # delay the next call's work-pool releases (ctxp bufs=4 holds
                # both calls' tiles)
                def norm(tail=False):
                    for half in range(2):
                        hp = slice(half * 64, half * 64 + 64)
                        rs = rpool.tile([64, 512], F32, tag="rs")
                        if tail:
                            # ACT is idle after the last exp: offload the
                            # denominator copies so the final norm chain
                            # isn't fully DVE-serial
                            nc.scalar.copy(rs[:], ctxs[half][64:128, :])
                        else:
                            nc.vector.tensor_copy(rs[:], ctxs[half][64:128, :])
                        rcp = rpool.tile([64, 512], F32, tag="rcp")
                        nc.vector.reciprocal_approx_fast(rcp[:], rs[:])
                        nc.vector.tensor_mul(
                            ctxT_sb[hp, p, q0:q0 + BT],
                            ctxs[half][0:HD, :],
                            rcp[:],
                        )
                return norm

            def out_tile0(n, t2b, tail=False):
                # all three pair-chunks into one PSUM tile, bf16 store.
                # Mid-attention the store trigger must NOT sit on the scalar
                # queue (it would stall the exp stream behind it); gpsimd is
                # idle so odd n goes there.  In the tail ACT is free: the
                # psum->sbuf copy moves to ACT and the store may use scalar.
                col0 = t2b * 512
                po = work.tile([128, 1024], F32, tag="work")
                for c in range(NPAIR):
                    nc.tensor.matmul(
                        po[:, 0:512],
                        wo_sb[:, c, n * 128:(n + 1) * 128],
                        ctxT_sb[:, c, col0:col0 + 512],
                        start=(c == 0), stop=(c == NPAIR - 1),
                    )
                ot = opool.tile([128, 512], BF16, tag="ot")
                if tail:
                    nc.scalar.copy(ot[:], po[:, 0:512])
                    eng = (nc.scalar, nc.gpsimd, nc.sync)[(n // 2) % 3]
                else:
                    nc.vector.tensor_copy(ot[:], po[:, 0:512])
                    eng = nc.sync if n % 2 == 0 else nc.gpsimd
                eng.dma_start(outT_d[n * 128:(n + 1) * 128, col0:col0 + 512],
                              ot[:])

            def out_stage01(n):
                # cols [1536,2048): pairs 0+1 partial -> fp32 SBUF stage
                # (runs inside the last attention call)
                po = work.tile([128, 1024], F32, tag="work")
                for c in (0, 1):
                    nc.tensor.matmul(
                        po[:, 0:512],
                        wo_sb[:, c, n * 128:(n + 1) * 128],
                        ctxT_sb[:, c, 1536:2048],
                        start=(c == 0), stop=(c == 1),
                    )
                nc.vector.tensor_copy(stage_sb[:, n, :], po[:, 0:512])

            def out_final2(n):
                # tail: pair-2 matmul + staged add -> bf16 store
                po = work.tile([128, 1024], F32, tag="work")
                nc.tensor.matmul(
                    po[:, 0:512],
                    wo_sb[:, 2, n * 128:(n + 1) * 128],
                    ctxT_sb[:, 2, 1536:2048],
                    start=True, stop=True,
                )
                ot = opool.tile([128, 512], BF16, tag="ot")
                nc.vector.tensor_add(out=ot[:], in0=po[:, 0:512],
                                     in1=stage_sb[:, n, :])
                eng = (nc.sync, nc.scalar, nc.gpsimd)[n % 3]
                eng.dma_start(outT_d[n * 128:(n + 1) * 128, 1536:2048],
                              ot[:])

            # ---- emission order -------------------------------------------
            import functools
            # first q/k tiles full-width: their first two matmuls gate on
            # the eb0 C01 piece alone, and one bias-add per tile beats the
            # DVE-serial chain of four 256-wide adds
            qk_tile(0, 0, 0)
            qk_tile(0, 1, 0)

            qk = lambda p, w, t: functools.partial(qk_tile, p, w, t)
            vp = lambda s: functools.partial(v_proj, s)
            ot0 = lambda n, t2b: functools.partial(out_tile0, n, t2b)
            st = lambda n: functools.partial(out_stage01, n)

            # t2-major: filler units placed so (a) q/k tiles precede their
            # consuming call, (b) v blocks precede the group that AVs them,
            # (c) out-proj of group g runs in the ACT-bound groups g+1/g+2,
            # (d) each call's norm runs as the first filler of the next call
            # "diagonal" order: larger-t2 (ACT-heavy) calls pulled as early
            # as their q/k tiles allow, so the exp engine ramps up during
            # the DMA-bound startup instead of idling through the small-t2
            # group; out-proj blocks still land in the late ACT-bound calls
            # q/k tiles for call i are emitted in call i-2 (one call of slack:
            # emitted-last-in-call-i-1 units were measured landing ~3us after
            # call i's scores wanted them), v blocks before the group that
            # AVs them, out-proj in the late ACT-bound calls
            calls = [
                (0, 0, [vp(0), vp(1), qk(1, 0, 0), qk(1, 1, 0),
                        vp(2), vp(3)], None),
                (1, 0, [qk(0, 0, 1), qk(0, 1, 1), vp(4), vp(5)], None),
                (0, 1, [qk(2, 0, 0), qk(2, 1, 0), qk(1, 0, 1), qk(1, 1, 1),
                        vp(6), vp(7)], None),
                (2, 0, [qk(0, 0, 2), qk(0, 1, 2), vp(8), vp(9)], None),
                (1, 1, [qk(2, 0, 1), qk(2, 1, 1), vp(10), vp(11)], None),
                (0, 2, [qk(1, 0, 2), qk(1, 1, 2), vp(12), vp(13)], None),
                (2, 1, [qk(0, 0, 3), qk(0, 1, 3), ot0(0, 0), ot0(1, 0)], None),
                (1, 2, [qk(2, 0, 2), qk(2, 1, 2), vp(14), vp(15),
                        ot0(2, 0)], None),
                (0, 3, [qk(1, 0, 3), qk(1, 1, 3), ot0(3, 0), ot0(4, 0)], None),
                (2, 2, [qk(2, 0, 3), qk(2, 1, 3), ot0(5, 0),
                        ot0(0, 1), ot0(1, 1)], None),
                (1, 3, [ot0(2, 1), ot0(3, 1), ot0(4, 1)], None),
                (2, 3, [ot0(5, 1), st(0), st(2), st(4),
                        ot0(0, 2), ot0(1, 2)], None),
            ]
            prev_norm = None
            for (p, t2, fills, late) in calls:
                if prev_norm is not None:
                    fills.insert(0, prev_norm)
                prev_norm = attention(p, t2, fills, late)
            # block-2 leftovers emitted here: they only need group-2 norms,
            # so they execute DURING the last call's final norm and keep the
            # PE clock from dropping before the tail matmuls
            for n in (2, 3, 4, 5):
                out_tile0(n, 2)
            prev_norm(tail=True)   # last call's norm, inline before the tail
            # tail: cols [1536,2048) — even n add the pair-2 matmul to the
            # staged pair-0/1 partial (DVE), odd n run the full 3-pair tile
            # with its copy on the now-idle ACT, balancing the tail engines
            for n in range(6):
                if n % 2 == 0:
                    out_final2(n)
                else:
                    out_tile0(n, 3, tail=True)
    nc.finalize()
    return nc


_NC_CACHE = None


def _get_nc():
    global _NC_CACHE
    if _NC_CACHE is None:
        _NC_CACHE = build_nc()
    return _NC_CACHE


def make_in_maps(x, Wq, Wk, Wv, bq, bk, bv, Wo, bo):
    bf16 = ml_dtypes.bfloat16
    f8 = ml_dtypes.float8_e4m3fn
    # tri[sk, sq] = 1 where sq >= sk (keep), 0 on causally-masked entries
    tri = (np.arange(128)[None, :] >= np.arange(128)[:, None]) \
        .astype(np.float32).astype(bf16)

    def pack_pairs8(w_all):
        # [D, 384] -> [NPAIR*128, 3*2*128]: pair p rows = [r, (C j m)]
        # with d = C*256 + j*128 + r  (DoubleRow K-subtile pairs)
        blks = []
        for p in range(NPAIR):
            blk = w_all[:, p * 128:(p + 1) * 128]          # [768, 128]
            blk = blk.reshape(3, 2, 128, 128).transpose(2, 0, 1, 3)
            blks.append(blk.reshape(128, 3 * 2 * 128))
        return np.ascontiguousarray(np.concatenate(blks, axis=0))

    in_maps = []
    for core in range(8):
        b, g = core // 2, core % 2
        hs = slice(6 * g, 6 * g + 6)
        xT64 = np.asarray(x[b]).T                          # [768, 2048]
        # [D, S] -> [8, 128, NC_D, 256] -> rows e*128+p hold [c, col]
        xT = xT64.reshape(NC_D, 128, 8, 256) \
            .transpose(2, 1, 0, 3).reshape(8 * 128, NC_D * 256)
        xT = np.ascontiguousarray(xT).astype(bf16)
        # fp8 copy: rows eb*128+p hold [C, j, 512], d = C*256 + j*128 + p
        xq8 = xT64.reshape(3, 2, 128, 4, 512) \
            .transpose(3, 2, 0, 1, 4).reshape(4 * 128, 3 * 2 * 512)
        xq8 = np.ascontiguousarray(xq8).astype(f8)
        wq_all = np.asarray(Wq[hs]).transpose(1, 0, 2).reshape(D, H * HD)
        wk_all = np.asarray(Wk[hs]).transpose(1, 0, 2).reshape(D, H * HD)
        wq8 = pack_pairs8(wq_all).astype(f8)
        wk8 = pack_pairs8(wk_all).astype(f8)
        wv_all = np.asarray(Wv[hs]).transpose(1, 0, 2).reshape(D, H * HD)
        wv = np.ascontiguousarray(
            wv_all.reshape(NC_D, 128, H * HD).transpose(1, 0, 2)
            .reshape(128, NC_D * H * HD)).astype(bf16)
        bqc = np.zeros((128, NPAIR), np.float32)
        bkc = np.zeros((128, NPAIR), np.float32)
        for p in range(NPAIR):
            bqc[0:64, p] = bq[6 * g + 2 * p]
            bqc[64:128, p] = bq[6 * g + 2 * p + 1]
            bkc[0:64, p] = bk[6 * g + 2 * p]
            bkc[64:128, p] = bk[6 * g + 2 * p + 1]
        bvr = np.ascontiguousarray(
            np.asarray(bv[hs]).reshape(1, H * HD)).astype(np.float32)
        wo_slice = np.asarray(Wo[384 * g:384 * (g + 1), :])   # [384, 768]
        wo = np.ascontiguousarray(
            wo_slice.reshape(NPAIR, 128, D).transpose(1, 0, 2)
            .reshape(128, NPAIR * D)).astype(bf16)
        in_maps.append({
            "xT": xT, "xq8": xq8, "wq8": wq8, "wk8": wk8, "wv": wv,
            "bq": bqc, "bk": bkc, "bv": bvr, "wo": wo,
            "tri": tri,
        })
    return in_maps


def gather_out(results, bo):
    out = np.empty((B, S, D), np.float32)
    bo32 = np.asarray(bo, np.float32)
    for b in range(B):
        pT = (results[2 * b]["outT"].astype(np.float32)
              + results[2 * b + 1]["outT"].astype(np.float32))
        out[b] = pT.T + bo32[None, :]
    return out


def kernel(x, Wq, Wk, Wv, bq, bk, bv, Wo, bo):
    from concourse.bass_utils import run_bass_kernel_spmd

    nc = _get_nc()
    in_maps = make_in_maps(x, Wq, Wk, Wv, bq, bk, bv, Wo, bo)
    res = run_bass_kernel_spmd(nc, in_maps, list(range(8)))
    return gather_out(res.results, bo)
